# revision 28
# baseline (speedup 1.0000x reference)
"""Trainium2 Bass kernel for nn_NDNRefinement (4-layer GNN message passing).

Strategy (8 NeuronCores):
- Sort triples by s_idx on host; shard triples so core c owns triples whose
  subject falls in its object range [c*OS, (c+1)*OS). Subject-side pooling is
  then core-local. Object-side rows are exchanged via a fixed-size-bucket
  AllToAll. Per-object MLPs are data-parallel over the object shard; the new
  object table is AllGathered between layers.
- Gathers use the Q7 dma_gather ucode instruction (batched, int16 indices):
  subject rows come feature-major (transpose mode) from the LOCAL table
  (ids < OSP fit int16); object rows are fetched as row PAIRS from the
  global table (pair id = gid>>1 fits int16) and combined with a parity
  outer-product + two vector selects. Pool-phase gathers run in entry mode,
  one instruction per tile side; recv rows >= 32768 are gathered from a
  DRAM->DRAM copied alias (recv_hi).
- Activations are kept feature-major through the MLP; W1b outputs are
  entry-major so pooling can use one-hot matmuls. Matmuls run in bf16 with
  fp32 PSUM. Layer-0 tables are padded to 128 features so gather rows are
  256B-aligned.
- All loop structure is static and identical across cores (SPMD); per-core
  raggedness is absorbed by host-computed padding (pad entries have one-hot
  loc -1 so they contribute zero to pooling).
"""

import os
import numpy as np
import ml_dtypes

import concourse.bass as bass
import concourse.bacc as bacc
import concourse.tile as tile
from concourse import mybir
from concourse.bass_utils import run_bass_kernel_spmd
from concourse.masks import make_identity

BF16 = mybir.dt.bfloat16
F32 = mybir.dt.float32
I32 = mybir.dt.int32
I16 = mybir.dt.int16
P = 128
NC = 8
ALPHA = 0.2
HI_BASE = 32768

# (din, h, dout) per layer
DIMS = [(64, 512, 128), (128, 512, 128), (128, 512, 128), (128, 128, 128)]


def _rup(x, m):
    return ((int(x) + m - 1) // m) * m


def _wrap16(vals):
    """ids (len = n*128) -> int16 [128, n*8]: v[i] at [i%16, i//16], x8 tiled."""
    n = len(vals)
    assert n % 128 == 0
    a = np.zeros((16, n // 16), np.int16)
    a[np.arange(n) % 16, np.arange(n) // 16] = np.asarray(vals, np.int16)
    return np.tile(a, (8, 1))


# ---------------------------------------------------------------------------
# Host preprocessing
# ---------------------------------------------------------------------------

def preprocess(inputs):
    """Compute the static schedule + per-core input maps from full inputs."""
    obj_vecs = np.asarray(inputs["obj_vecs"], np.float32)
    pred_vecs = np.asarray(inputs["pred_vecs"], np.float32)
    pred_boxes = np.asarray(inputs["pred_boxes"], np.float32)
    s_idx = np.asarray(inputs["s_idx"], np.int32)
    o_idx = np.asarray(inputs["o_idx"], np.int32)

    O = obj_vecs.shape[0]
    assert O % NC == 0
    OS = O // NC                      # real objects per core
    OSP = _rup(OS, P)                 # padded objects per core
    NT = OSP // P                     # object tiles per core
    OG = NC * OSP                     # padded global object count

    def gmap(idx):
        return ((idx // OS) * OSP + (idx % OS)).astype(np.int32)

    order = np.argsort(s_idx, kind="stable")
    s_sorted = s_idx[order]
    o_sorted = o_idx[order]
    bnd = np.searchsorted(s_sorted, np.arange(NC + 1) * OS)
    counts_c = np.diff(bnd)
    T_PC = max(_rup(counts_c.max(), 512), 512)
    NB = T_PC // 512

    # per-object counts (same every layer)
    cnt = np.bincount(s_idx, minlength=O) + np.bincount(o_idx, minlength=O)
    inv_cnt = (1.0 / np.maximum(cnt, 1)).astype(np.float32)

    percore = []
    maxB = 0
    for c in range(NC):
        sl = slice(bnd[c], bnd[c + 1])
        idxs = order[sl]
        n = len(idxs)
        s_c = s_sorted[sl]
        o_c = o_sorted[sl]
        d_c = (o_c // OS).astype(np.int64)

        # bucket ranks: entries sorted by (dest, o)
        ordb = np.lexsort((o_c, d_c))
        rank = np.empty(n, np.int64)
        d_srt = d_c[ordb]
        first = np.searchsorted(d_srt, np.arange(NC))
        rank[ordb] = np.arange(n) - first[d_srt]
        bc = np.bincount(d_c, minlength=NC)
        maxB = max(maxB, int(bc.max()) if n else 0)
        percore.append(dict(idxs=idxs, n=n, s_c=s_c, o_c=o_c, d_c=d_c,
                            rank=rank, bc=bc))

    # each bucket keeps headroom at the top for pad-entry trash writes
    n_pad_max = max(int(T_PC - pc["n"]) for pc in percore)
    S_B = _rup(maxB + n_pad_max // NC + 2, P)
    HB = min(HI_BASE, NC * S_B)          # recv rows >= HB go through recv_hi
    RECV_HI = max(NC * S_B - HB, P)

    # pool schedules; compute PS / PO_LO / PO_HI as max over cores
    PS = 1
    PO_LO = 1
    PO_HI = 1
    pool_lists = []
    for c in range(NC):
        pc = percore[c]
        s_loc = pc["s_c"] - c * OS
        tstart = np.searchsorted(s_loc, np.arange(NT + 1) * P)
        rows_all, locs_all = [], []
        for d in range(NC):
            qc = percore[d]
            m = qc["d_c"] == c
            rows_all.append(d * S_B + qc["rank"][m])
            locs_all.append(qc["o_c"][m] - c * OS)
        rows_all = np.concatenate(rows_all)
        locs_all = np.concatenate(locs_all)
        tsel = locs_all // P
        tiles = []
        for t in range(NT):
            a, b = int(tstart[t]), int(tstart[t + 1])
            sids = np.arange(a, b, dtype=np.int64)
            slocs = (s_loc[a:b] - t * P).astype(np.int64)
            m = tsel == t
            orows = rows_all[m]
            olocs = locs_all[m] - t * P
            lo = orows < HB
            tiles.append((sids, slocs, orows[lo], olocs[lo],
                          orows[~lo] - HB, olocs[~lo]))
            PS = max(PS, -(-len(sids) // P))
            PO_LO = max(PO_LO, -(-len(orows[lo]) // P))
            PO_HI = max(PO_HI, max(1, -(-len(orows[~lo]) // P)))
        pool_lists.append(tiles)
    NPK = PS + PO_LO + PO_HI

    cfg = dict(O=O, OS=OS, OSP=OSP, NT=NT, OG=OG, T_PC=T_PC, NB=NB,
               S_B=S_B, PS=PS, PO_LO=PO_LO, PO_HI=PO_HI, RECV_HI=RECV_HI,
               HB=HB)

    # ---- weights, shared across cores ----
    bf = ml_dtypes.bfloat16
    shared = {}
    shared["w_emb"] = np.asarray(inputs["W_emb"], np.float32).astype(bf)
    shared["b_emb"] = np.asarray(inputs["b_emb"], np.float32).reshape(-1, 1)
    shared["ones1"] = np.ones((1, P), bf)
    for li, (din, h, dout) in enumerate(DIMS):
        b1b = np.asarray(inputs[f"b1b{li}"], np.float32)
        w1a = np.asarray(inputs[f"W1a{li}"], np.float32)
        # pad s/o chunks of W1a to K=128 (tables are 128-wide)
        w1a_s = np.zeros((P, h), np.float32)
        w1a_s[:din] = w1a[:din]
        w1a_o = np.zeros((P, h), np.float32)
        w1a_o[:din] = w1a[2 * din:]
        shared[f"w1as{li}"] = w1a_s.astype(bf)
        shared[f"w1ap{li}"] = w1a[din:2 * din].astype(bf)
        shared[f"w1ao{li}"] = w1a_o.astype(bf)
        shared[f"w1b{li}"] = np.asarray(inputs[f"W1b{li}"], np.float32).astype(bf)
        shared[f"w2a{li}"] = np.asarray(inputs[f"W2a{li}"], np.float32).astype(bf)
        shared[f"w2b{li}"] = np.asarray(inputs[f"W2b{li}"], np.float32).astype(bf)
        shared[f"b1a{li}"] = np.asarray(inputs[f"b1a{li}"], np.float32).reshape(-1, P).T.copy()
        shared[f"b1bp{li}"] = b1b[h:h + dout].reshape(-1, 1).copy()
        shared[f"b1bs{li}"] = np.broadcast_to(b1b[:h].astype(bf), (P, h)).copy()
        shared[f"b1bo{li}"] = np.broadcast_to(b1b[h + dout:].astype(bf), (P, h)).copy()
        shared[f"b2a{li}"] = np.asarray(inputs[f"b2a{li}"], np.float32).reshape(-1, P).T.copy()
        shared[f"b2b{li}"] = np.asarray(inputs[f"b2b{li}"], np.float32).reshape(-1, 1).copy()
    shared["wbb"] = np.asarray(inputs["W_bb"], np.float32).astype(bf)
    shared["bbb"] = np.asarray(inputs["b_bb"], np.float32).reshape(-1, 1)

    # ---- per-core arrays ----
    x_full = np.concatenate([obj_vecs, pred_boxes], axis=1)       # (O, 68)

    in_maps = []
    for c in range(NC):
        pc = percore[c]
        idxs, n = pc["idxs"], pc["n"]
        m = {}
        xT = np.zeros((68, OSP), bf)
        xT[:, :OS] = x_full[c * OS:(c + 1) * OS].T.astype(bf)
        m["xt"] = xT
        pT = np.zeros((64, T_PC), bf)
        pT[:, :n] = pred_vecs[idxs].T.astype(bf)
        m["pred0"] = pT

        # s ids: local row ids (pad -> 0)
        sl_ = np.zeros((T_PC,), np.int64)
        sl_[:n] = pc["s_c"] - c * OS
        # o pair ids + parity over global padded ids (pad -> 0)
        og_ = np.zeros((T_PC,), np.int64)
        og_[:n] = gmap(pc["o_c"])
        opair = og_ >> 1
        par_ = (og_ & 1).astype(np.float32)
        m["parb"] = par_.astype(bf).reshape(1, T_PC)
        sgog = np.zeros((P, NB * 64), np.int16)
        for j in range(NB):
            blk = slice(j * 512, (j + 1) * 512)
            sgog[:, j * 64:j * 64 + 32] = _wrap16(sl_[blk])
            sgog[:, j * 64 + 32:j * 64 + 64] = _wrap16(opair[blk])
        m["sgog"] = sgog

        # bucket scatter positions; pad entries spread over per-bucket headroom
        ob_ = np.empty((T_PC,), np.int32)
        ob_[:n] = (pc["d_c"] * S_B + pc["rank"]).astype(np.int32)
        npad = T_PC - n
        if npad:
            i = np.arange(npad)
            d = i % NC
            slot = S_B - 1 - (i // NC)
            assert (slot >= pc["bc"][d]).all(), "trash slots collide with data"
            ob_[n:] = (d * S_B + slot).astype(np.int32)
        obi = np.zeros((P, NB * 4), np.int32)
        for j in range(NB):
            obi[:, j * 4:(j + 1) * 4] = ob_[j * 512:(j + 1) * 512].reshape(4, P).T
        m["obi"] = obi

        # pool schedule: ids int16 (16-wrapped), locs int32
        pool16 = np.zeros((P, NT * NPK * 8), np.int16)
        polocs = np.full((P, NT * NPK), -1, np.int32)

        def pack(ids, locs, want, pad_id=0):
            k = len(ids)
            i_ = np.full((want * P,), pad_id, np.int64)
            l_ = np.full((want * P,), -1, np.int64)
            i_[:k] = ids
            l_[:k] = locs
            return _wrap16(i_), l_.reshape(want, P).T

        for t in range(NT):
            sids, slocs, lor, lol, hir, hil = pool_lists[c][t]
            i16, l32 = pack(sids, slocs, PS)
            pool16[:, (t * NPK) * 8:(t * NPK + PS) * 8] = i16
            polocs[:, t * NPK:t * NPK + PS] = l32
            i16, l32 = pack(lor, lol, PO_LO, pad_id=S_B - 1)
            pool16[:, (t * NPK + PS) * 8:(t * NPK + PS + PO_LO) * 8] = i16
            polocs[:, t * NPK + PS:t * NPK + PS + PO_LO] = l32
            i16, l32 = pack(hir, hil, PO_HI, pad_id=RECV_HI - 1)
            pool16[:, (t * NPK + PS + PO_LO) * 8:(t * NPK + NPK) * 8] = i16
            polocs[:, t * NPK + PS + PO_LO:t * NPK + NPK] = l32
        m["pool16"] = pool16
        m["polocs"] = polocs

        iv = np.zeros((OSP,), np.float32)
        iv[:OS] = inv_cnt[c * OS:(c + 1) * OS]
        m["invc"] = iv.reshape(NT, P).T.copy()
        m.update(shared)
        in_maps.append(m)

    return cfg, in_maps


# ---------------------------------------------------------------------------
# Kernel builder
# ---------------------------------------------------------------------------

def build_kernel(cfg):
    OSP, NT, OG = cfg["OSP"], cfg["NT"], cfg["OG"]
    T_PC, NB, S_B = cfg["T_PC"], cfg["NB"], cfg["S_B"]
    PS, PO_LO, PO_HI = cfg["PS"], cfg["PO_LO"], cfg["PO_HI"]
    RECV_HI, HB = cfg["RECV_HI"], cfg["HB"]
    NPK = PS + PO_LO + PO_HI

    nc = bacc.Bacc("TRN2", target_bir_lowering=False, debug=False,
                   num_devices=NC)

    # ---- parameters ----
    xt = nc.declare_dram_parameter("xt", [68, OSP], BF16, isOutput=False)
    pred0 = nc.declare_dram_parameter("pred0", [64, T_PC], BF16, isOutput=False)
    sgog = nc.declare_dram_parameter("sgog", [P, NB * 64], I16, isOutput=False)
    obi_p = nc.declare_dram_parameter("obi", [P, NB * 4], I32, isOutput=False)
    parb = nc.declare_dram_parameter("parb", [1, T_PC], BF16, isOutput=False)
    pool16 = nc.declare_dram_parameter("pool16", [P, NT * NPK * 8], I16, isOutput=False)
    polocs = nc.declare_dram_parameter("polocs", [P, NT * NPK], I32, isOutput=False)
    invc = nc.declare_dram_parameter("invc", [P, NT], F32, isOutput=False)
    ones1 = nc.declare_dram_parameter("ones1", [1, P], BF16, isOutput=False)

    w_emb = nc.declare_dram_parameter("w_emb", [68, 64], BF16, isOutput=False)
    b_emb = nc.declare_dram_parameter("b_emb", [64, 1], F32, isOutput=False)
    wp = {}
    for li, (din, h, dout) in enumerate(DIMS):
        wp[f"w1as{li}"] = nc.declare_dram_parameter(f"w1as{li}", [P, h], BF16, isOutput=False)
        wp[f"w1ap{li}"] = nc.declare_dram_parameter(f"w1ap{li}", [din, h], BF16, isOutput=False)
        wp[f"w1ao{li}"] = nc.declare_dram_parameter(f"w1ao{li}", [P, h], BF16, isOutput=False)
        wp[f"w1b{li}"] = nc.declare_dram_parameter(f"w1b{li}", [h, 2 * h + dout], BF16, isOutput=False)
        wp[f"w2a{li}"] = nc.declare_dram_parameter(f"w2a{li}", [h, h], BF16, isOutput=False)
        wp[f"w2b{li}"] = nc.declare_dram_parameter(f"w2b{li}", [h, dout], BF16, isOutput=False)
        wp[f"b1a{li}"] = nc.declare_dram_parameter(f"b1a{li}", [P, h // P], F32, isOutput=False)
        wp[f"b1bp{li}"] = nc.declare_dram_parameter(f"b1bp{li}", [dout, 1], F32, isOutput=False)
        wp[f"b1bs{li}"] = nc.declare_dram_parameter(f"b1bs{li}", [P, h], BF16, isOutput=False)
        wp[f"b1bo{li}"] = nc.declare_dram_parameter(f"b1bo{li}", [P, h], BF16, isOutput=False)
        wp[f"b2a{li}"] = nc.declare_dram_parameter(f"b2a{li}", [P, h // P], F32, isOutput=False)
        wp[f"b2b{li}"] = nc.declare_dram_parameter(f"b2b{li}", [dout, 1], F32, isOutput=False)
    wbb = nc.declare_dram_parameter("wbb", [P, 4], BF16, isOutput=False)
    bbb = nc.declare_dram_parameter("bbb", [4, 1], F32, isOutput=False)

    out = nc.declare_dram_parameter("out", [4, OSP], F32, isOutput=True)

    # ---- internal DRAM ----
    # tables are 128-wide on every layer (layer-0 zero-padded)
    tabs = [nc.dram_tensor(f"tab{li}", [OG, P], BF16, addr_space="Shared")
            for li in range(4)]
    agins = [nc.dram_tensor(f"agin{li}", [OSP, P], BF16) for li in range(4)]
    preds = [pred0]
    for li in range(1, 4):
        preds.append(nc.dram_tensor(f"pred{li}", [P, T_PC], BF16))
    stages, sends, recvs, recv_his = [], [], [], []
    for li, (din, h, dout) in enumerate(DIMS):
        stages.append(nc.dram_tensor(f"stage{li}", [T_PC, h], BF16))
        sends.append(nc.dram_tensor(f"send{li}", [NC * S_B, h], BF16))
        recvs.append(nc.dram_tensor(f"recv{li}", [NC * S_B, h], BF16))
        recv_his.append(nc.dram_tensor(f"recvhi{li}", [RECV_HI, h], BF16))

    PRELU = mybir.ActivationFunctionType.Prelu
    COPY = mybir.ActivationFunctionType.Copy
    GRPS = [list(range(NC))]

    import contextlib
    _PROF = bool(int(os.environ.get("KPROF", "0")))

    with tile.TileContext(nc) as tc:
        def scope(name):
            return tc.spectator_scope(name) if _PROF else contextlib.nullcontext()

        with tc.tile_pool(name="cst", bufs=1) as cst:
            # constants
            ident = cst.tile([P, P], F32)
            make_identity(nc, ident[:])
            ident_bf = cst.tile([P, P], BF16)
            nc.vector.tensor_copy(out=ident_bf[:], in_=ident[:])
            iota = cst.tile([P, P], I32)
            nc.gpsimd.iota(iota[:], pattern=[[1, P]], base=0, channel_multiplier=0)
            zrow = cst.tile([1, 512], BF16)
            nc.vector.memset(zrow[:], 0.0)

            W = {}

            def load_w(name, src_ap, hh, ww, dt):
                t = cst.tile([hh, ww], dt, tag=name)
                nc.sync.dma_start(out=t[:], in_=src_ap)
                W[name] = t

            load_w("w_emb", w_emb[:, :], 68, 64, BF16)
            load_w("b_emb", b_emb[:, :], 64, 1, F32)
            load_w("wbb", wbb[:, :], P, 4, BF16)
            load_w("bbb", bbb[:, :], 4, 1, F32)
            load_w("invc", invc[:, :], P, NT, F32)
            load_w("ones1", ones1[:, :], 1, P, BF16)
            load_w("parb", parb[:, :], 1, T_PC, BF16)
            for li, (din, h, dout) in enumerate(DIMS):
                load_w(f"w1as{li}", wp[f"w1as{li}"][:, :], P, h, BF16)
                load_w(f"w1ap{li}", wp[f"w1ap{li}"][:, :], din, h, BF16)
                load_w(f"w1ao{li}", wp[f"w1ao{li}"][:, :], P, h, BF16)
                for k in range(h // P):
                    load_w(f"w1b{li}_{k}", wp[f"w1b{li}"][k * P:(k + 1) * P, :],
                           P, 2 * h + dout, BF16)
                    load_w(f"w2a{li}_{k}", wp[f"w2a{li}"][k * P:(k + 1) * P, :],
                           P, h, BF16)
                    load_w(f"w2b{li}_{k}", wp[f"w2b{li}"][k * P:(k + 1) * P, :],
                           P, dout, BF16)
                load_w(f"b1a{li}", wp[f"b1a{li}"][:, :], P, h // P, F32)
                load_w(f"b1bp{li}", wp[f"b1bp{li}"][:, :], dout, 1, F32)
                load_w(f"b1bs{li}", wp[f"b1bs{li}"][:, :], P, h, BF16)
                load_w(f"b1bo{li}", wp[f"b1bo{li}"][:, :], P, h, BF16)
                load_w(f"b2a{li}", wp[f"b2a{li}"][:, :], P, h // P, F32)
                load_w(f"b2b{li}", wp[f"b2b{li}"][:, :], dout, 1, F32)

            # ---------------- embedding phase ----------------
            NEB = -(-OSP // 512)
            with (
                scope("emb"),
                tc.tile_pool(name="esb", bufs=3) as esb,
                tc.tile_pool(name="eps", bufs=3, space="PSUM") as eps,
            ):
                for b in range(NEB):
                    c0 = b * 512
                    w = min(512, OSP - c0)
                    xin = esb.tile([68, 512], BF16, tag="xin")
                    nc.sync.dma_start(out=xin[:, :w], in_=xt[:, c0:c0 + w])
                    pse = eps.tile([64, 512], F32, space="PSUM", tag="pse")
                    nc.tensor.matmul(out=pse[:, :w], lhsT=W["w_emb"][:], rhs=xin[:, :w],
                                     start=True, stop=True)
                    ebt = esb.tile([64, 512], BF16, tag="ebt")
                    nc.scalar.activation(out=ebt[:, :w], in_=pse[:, :w], func=PRELU,
                                         bias=W["b_emb"][:, :1], alpha=ALPHA)
                    for q in range(-(-w // P)):
                        qw = min(P, w - q * P)
                        ptr = eps.tile([P, 64], BF16, space="PSUM", tag="ptr")
                        nc.tensor.transpose(out=ptr[:qw, :], in_=ebt[:, q * P:q * P + qw],
                                            identity=ident_bf[:64, :64])
                        ent = esb.tile([P, P], BF16, tag="ent")
                        nc.vector.tensor_copy(out=ent[:qw, :64], in_=ptr[:qw, :])
                        nc.vector.memset(ent[:, 64:], 0.0)
                        nc.sync.dma_start(out=agins[0][c0 + q * P:c0 + q * P + qw, :],
                                          in_=ent[:qw, :])
            with scope("ag_emb"):
                nc.gpsimd.collective_compute(
                    "AllGather", mybir.AluOpType.bypass, replica_groups=GRPS,
                    ins=[agins[0][:]], outs=[tabs[0][:]])

            # ---------------- layers ----------------
            _MAXL = int(os.environ.get("KMAXL", "4"))
            for li, (din, h, dout) in enumerate(DIMS[:_MAXL]):
                tab_in = tabs[li]
                agin_in = agins[li]
                pred_in = preds[li]
                stage, send, recv = stages[li], sends[li], recvs[li]
                recv_hi = recv_his[li]
                NH = h // P
                s_cols = (0, h)
                p_cols = (h, h + dout)
                o_cols = (h + dout, 2 * h + dout)
                tab_pair = tab_in[:, :].rearrange("(a b) d -> a (b d)", b=2)

                # ---- phase A: triple MLP ----
                with (
                    scope(f"A{li}"),
                    tc.tile_pool(name=f"asb{li}", bufs=3) as asb,
                    tc.tile_pool(name=f"apshid{li}", bufs=NH, space="PSUM") as aps_hid,
                    tc.tile_pool(name=f"apsout{li}", bufs=3, space="PSUM") as aps_out,
                    tc.tile_pool(name=f"apspar{li}", bufs=1, space="PSUM") as aps_par,
                ):
                    for j in range(NB):
                        idx16 = asb.tile([P, 64], I16, tag="idx16")
                        nc.sync.dma_start(out=idx16[:], in_=sgog[:, 64 * j:64 * j + 64])
                        obit = asb.tile([P, 4], I32, tag="obit")
                        nc.sync.dma_start(out=obit[:], in_=obi_p[:, 4 * j:4 * j + 4])

                        sT3 = asb.tile([P, 1, 512], BF16, tag="sT3")
                        nc.gpsimd.dma_gather(
                            out_ap=sT3[:], in_ap=agin_in[:, :],
                            idxs_ap=idx16[:, 0:32], num_idxs=512,
                            num_idxs_reg=512, elem_size=P, transpose=True)
                        oPair = asb.tile([P, 2, 512], BF16, tag="oPair")
                        nc.gpsimd.dma_gather(
                            out_ap=oPair[:], in_ap=tab_pair,
                            idxs_ap=idx16[:, 32:64], num_idxs=512,
                            num_idxs_reg=512, elem_size=2 * P, transpose=True)
                        # parity combine: oT = oPair0 + (oPair1-oPair0)*par
                        parp = aps_par.tile([P, 512], F32, space="PSUM", tag="parp")
                        nc.tensor.matmul(out=parp[:], lhsT=W["ones1"][:],
                                         rhs=W["parb"][:, 512 * j:512 * (j + 1)],
                                         start=True, stop=True)
                        dT = asb.tile([P, 512], BF16, tag="dT")
                        nc.vector.tensor_tensor(out=dT[:], in0=oPair[:, 1, :],
                                                in1=oPair[:, 0, :],
                                                op=mybir.AluOpType.subtract)
                        dTp = asb.tile([P, 512], BF16, tag="dTp")
                        nc.vector.tensor_tensor(out=dTp[:], in0=dT[:], in1=parp[:],
                                                op=mybir.AluOpType.mult)
                        oT = asb.tile([P, 512], BF16, tag="oT")
                        nc.vector.tensor_tensor(out=oT[:], in0=oPair[:, 0, :],
                                                in1=dTp[:], op=mybir.AluOpType.add)

                        pT = asb.tile([din, 512], BF16, tag="pT")
                        nc.sync.dma_start(out=pT[:], in_=pred_in[:din, 512 * j:512 * (j + 1)])

                        # hid
                        hidT = []
                        for mh in range(NH):
                            ph = aps_hid.tile([P, 512], F32, space="PSUM", tag="ph")
                            nc.tensor.matmul(
                                out=ph[:], lhsT=W[f"w1as{li}"][:, mh * P:(mh + 1) * P],
                                rhs=sT3[:, 0, :], start=True, stop=False)
                            nc.tensor.matmul(
                                out=ph[:], lhsT=W[f"w1ap{li}"][:, mh * P:(mh + 1) * P],
                                rhs=pT[:], start=False, stop=False)
                            nc.tensor.matmul(
                                out=ph[:], lhsT=W[f"w1ao{li}"][:, mh * P:(mh + 1) * P],
                                rhs=oT[:], start=False, stop=True)
                            ht = asb.tile([P, 512], BF16, tag=f"hidT{mh}",
                                          name=f"hidT{mh}")
                            nc.scalar.activation(out=ht[:], in_=ph[:], func=PRELU,
                                                 bias=W[f"b1a{li}"][:, mh:mh + 1],
                                                 alpha=ALPHA)
                            hidT.append(ht)

                        # new_s / new_o (entry-major), leaky on vector
                        for (cols, bname, is_s) in ((s_cols, f"b1bs{li}", True),
                                                    (o_cols, f"b1bo{li}", False)):
                            ovb = asb.tile([P, 4, h], BF16,
                                           tag="svb" if is_s else "ovb")
                            for e in range(4):
                                po = aps_out.tile([P, 512], F32, space="PSUM", tag="po")
                                for k in range(NH):
                                    nc.tensor.matmul(
                                        out=po[:, :h],
                                        lhsT=hidT[k][:, e * P:(e + 1) * P],
                                        rhs=W[f"w1b{li}_{k}"][:, cols[0]:cols[1]],
                                        start=(k == 0), stop=(k == NH - 1))
                                nc.vector.tensor_tensor(
                                    out=po[:, :h], in0=po[:, :h], in1=W[bname][:],
                                    op=mybir.AluOpType.add)
                                tl = asb.tile([P, 512], BF16, tag="tleak")
                                nc.vector.tensor_scalar(
                                    out=tl[:, :h], in0=po[:, :h], scalar1=ALPHA,
                                    scalar2=None, op0=mybir.AluOpType.mult)
                                nc.vector.tensor_tensor(
                                    out=ovb[:, e, :], in0=po[:, :h], in1=tl[:, :h],
                                    op=mybir.AluOpType.max)
                            if is_s:
                                for e in range(4):
                                    r0 = 512 * j + e * P
                                    nc.sync.dma_start(out=stage[r0:r0 + P, :],
                                                      in_=ovb[:, e, :])
                            else:
                                for e in range(4):
                                    nc.gpsimd.indirect_dma_start(
                                        out=send[:],
                                        out_offset=bass.IndirectOffsetOnAxis(
                                            ap=obit[:, e:e + 1], axis=0),
                                        in_=ovb[:, e, :], in_offset=None)

                        # new_p (feature-major), not needed after last layer
                        if li < 3:
                            pp = aps_out.tile([P, 512], F32, space="PSUM", tag="po")
                            for k in range(NH):
                                nc.tensor.matmul(
                                    out=pp[:dout, :],
                                    lhsT=W[f"w1b{li}_{k}"][:, p_cols[0]:p_cols[1]],
                                    rhs=hidT[k][:],
                                    start=(k == 0), stop=(k == NH - 1))
                            pv = asb.tile([dout, 512], BF16, tag="pv")
                            nc.scalar.activation(out=pv[:], in_=pp[:dout, :], func=PRELU,
                                                 bias=W[f"b1bp{li}"][:, :1], alpha=ALPHA)
                            nc.sync.dma_start(
                                out=preds[li + 1][:, 512 * j:512 * (j + 1)], in_=pv[:])

                # ---- phase B: AllToAll + recv_hi alias copy ----
                with scope(f"B{li}"):
                    nc.gpsimd.collective_compute(
                        "AllToAll", mybir.AluOpType.bypass, replica_groups=GRPS,
                        ins=[send[:]], outs=[recv[:]])
                    if NC * S_B > HB:
                        nc.sync.dma_start(out=recv_hi[:, :], in_=recv[HB:, :])
                    # guaranteed-finite rows for pool-pad gathers (top slot of
                    # bucket 0 resp. bucket 7 is never real data)
                    nc.sync.dma_start(out=recv[S_B - 1:S_B, :], in_=zrow[:, :h])
                    nc.sync.dma_start(out=recv_hi[RECV_HI - 1:RECV_HI, :],
                                      in_=zrow[:, :h])

                # ---- phase C: pooling + object MLP ----
                with (
                    scope(f"C{li}"),
                    tc.tile_pool(name=f"csb{li}", bufs=3) as csb,
                    tc.tile_pool(name=f"cpool{li}", bufs=2, space="PSUM") as cps_pool,
                    tc.tile_pool(name=f"ctr{li}", bufs=2, space="PSUM") as cps_tr,
                    tc.tile_pool(name=f"cmlp{li}", bufs=2, space="PSUM") as cps_mlp,
                ):
                    ng = -(-NT // 4)
                    for grp in range(ng):
                        t0 = grp * 4
                        tn = min(4, NT - t0)
                        gw = tn * P
                        pooledT = [csb.tile([P, 512], BF16, tag=f"pooledT{k}",
                                            name=f"pooledT{k}")
                                   for k in range(NH)]
                        for tt in range(t0, t0 + tn):
                            pps = cps_pool.tile([P, h], F32, space="PSUM", tag="pps")
                            ilt16 = csb.tile([P, NPK * 8], I16, tag="ilt16")
                            nc.sync.dma_start(
                                out=ilt16[:],
                                in_=pool16[:, tt * NPK * 8:(tt + 1) * NPK * 8])
                            iltl = csb.tile([P, NPK], I32, tag="iltl")
                            nc.sync.dma_start(
                                out=iltl[:], in_=polocs[:, tt * NPK:(tt + 1) * NPK])
                            svals = csb.tile([P, PS, h], BF16, tag="svals")
                            nc.gpsimd.dma_gather(
                                out_ap=svals[:], in_ap=stage[:, :],
                                idxs_ap=ilt16[:, 0:PS * 8], num_idxs=PS * P,
                                num_idxs_reg=PS * P, elem_size=h, transpose=False)
                            ovlo = csb.tile([P, PO_LO, h], BF16, tag="ovlo")
                            nc.gpsimd.dma_gather(
                                out_ap=ovlo[:], in_ap=recv[:, :],
                                idxs_ap=ilt16[:, PS * 8:(PS + PO_LO) * 8],
                                num_idxs=PO_LO * P, num_idxs_reg=PO_LO * P,
                                elem_size=h, transpose=False)
                            ovhi = csb.tile([P, PO_HI, h], BF16, tag="ovhi")
                            nc.gpsimd.dma_gather(
                                out_ap=ovhi[:], in_ap=recv_hi[:, :],
                                idxs_ap=ilt16[:, (PS + PO_LO) * 8:NPK * 8],
                                num_idxs=PO_HI * P, num_idxs_reg=PO_HI * P,
                                elem_size=h, transpose=False)
                            nmm = 0
                            for (vals, PN, base) in ((svals, PS, 0),
                                                     (ovlo, PO_LO, PS),
                                                     (ovhi, PO_HI, PS + PO_LO)):
                                for k in range(PN):
                                    lcol = base + k
                                    oh = csb.tile([P, P], BF16, tag="oh")
                                    nc.vector.tensor_tensor(
                                        out=oh[:],
                                        in0=iltl[:, lcol:lcol + 1].to_broadcast([P, P]),
                                        in1=iota[:], op=mybir.AluOpType.is_equal)
                                    nc.tensor.matmul(out=pps[:], lhsT=oh[:],
                                                     rhs=vals[:, k, :],
                                                     start=(nmm == 0),
                                                     stop=(nmm == NPK - 1))
                                    nmm += 1
                            pob = csb.tile([P, h], BF16, tag="pob")
                            nc.scalar.activation(out=pob[:], in_=pps[:], func=COPY,
                                                 scale=W["invc"][:, tt:tt + 1])
                            for k in range(NH):
                                ptr2 = cps_tr.tile([P, P], BF16, space="PSUM", tag="ptr2")
                                nc.tensor.transpose(out=ptr2[:],
                                                    in_=pob[:, k * P:(k + 1) * P],
                                                    identity=ident_bf[:])
                                nc.vector.tensor_copy(
                                    out=pooledT[k][:, (tt - t0) * P:(tt - t0 + 1) * P],
                                    in_=ptr2[:])
                        # object MLP on gw objects
                        hid2 = []
                        for mh in range(NH):
                            p2 = cps_mlp.tile([P, 512], F32, space="PSUM", tag="p2")
                            for k in range(NH):
                                nc.tensor.matmul(
                                    out=p2[:, :gw],
                                    lhsT=W[f"w2a{li}_{k}"][:, mh * P:(mh + 1) * P],
                                    rhs=pooledT[k][:, :gw],
                                    start=(k == 0), stop=(k == NH - 1))
                            h2 = csb.tile([P, 512], BF16, tag=f"h2_{mh}",
                                          name=f"h2_{mh}")
                            nc.scalar.activation(out=h2[:, :gw], in_=p2[:, :gw],
                                                 func=PRELU,
                                                 bias=W[f"b2a{li}"][:, mh:mh + 1],
                                                 alpha=ALPHA)
                            hid2.append(h2)
                        pno = cps_mlp.tile([P, 512], F32, space="PSUM", tag="p2")
                        for k in range(NH):
                            nc.tensor.matmul(out=pno[:dout, :gw],
                                             lhsT=W[f"w2b{li}_{k}"][:],
                                             rhs=hid2[k][:, :gw],
                                             start=(k == 0), stop=(k == NH - 1))
                        noT = csb.tile([dout, 512], BF16, tag="noT")
                        nc.scalar.activation(out=noT[:, :gw], in_=pno[:dout, :gw],
                                             func=PRELU, bias=W[f"b2b{li}"][:, :1],
                                             alpha=ALPHA)
                        if li < 3:
                            for q in range(tn):
                                ptr3 = cps_tr.tile([P, P], BF16, space="PSUM", tag="ptr2")
                                nc.tensor.transpose(out=ptr3[:, :dout],
                                                    in_=noT[:, q * P:(q + 1) * P],
                                                    identity=ident_bf[:])
                                ent2 = csb.tile([P, P], BF16, tag="ent2")
                                nc.vector.tensor_copy(out=ent2[:, :dout],
                                                      in_=ptr3[:, :dout])
                                r0 = (t0 + q) * P
                                nc.sync.dma_start(out=agins[li + 1][r0:r0 + P, :],
                                                  in_=ent2[:])
                        else:
                            phd = cps_mlp.tile([4, 512], F32, space="PSUM", tag="phd")
                            nc.tensor.matmul(out=phd[:, :gw], lhsT=W["wbb"][:],
                                             rhs=noT[:, :gw], start=True, stop=True)
                            ho = csb.tile([4, 512], F32, tag="ho")
                            nc.scalar.activation(out=ho[:, :gw], in_=phd[:, :gw],
                                                 func=PRELU, bias=W["bbb"][:, :1],
                                                 alpha=ALPHA)
                            nc.sync.dma_start(out=out[:, t0 * P:t0 * P + gw],
                                              in_=ho[:, :gw])

                # ---- phase D: AllGather new object table ----
                if li < 3:
                    with scope(f"D{li}"):
                        nc.gpsimd.collective_compute(
                            "AllGather", mybir.AluOpType.bypass, replica_groups=GRPS,
                            ins=[agins[li + 1][:]], outs=[tabs[li + 1][:]])

    nc.compile()
    return nc


# ---------------------------------------------------------------------------
# Entry point
# ---------------------------------------------------------------------------

_CACHE = {}


def kernel(**inputs) -> np.ndarray:
    cfg, in_maps = preprocess(inputs)
    key = tuple(sorted(cfg.items()))
    if key not in _CACHE:
        _CACHE[key] = build_kernel(cfg)
    nc = _CACHE[key]
    res = run_bass_kernel_spmd(nc, in_maps, list(range(NC)))
    O, OS = cfg["O"], cfg["OS"]
    full = np.zeros((4, O), np.float32)
    for c in range(NC):
        full[:, c * OS:(c + 1) * OS] = res.results[c]["out"][:, :OS]
    return np.ascontiguousarray(full.T)
